# revision 1
# baseline (speedup 1.0000x reference)
"""Llama GQA attention (B=1, Q=1024, PAST=3072, HID=4096, NH=32, NKV=8, HD=128)
tensor-parallel over heads across 8 NeuronCores.

Per core c: kv head c, query heads 4c..4c+3. Each core computes its partial
o_proj contribution [1024, 4096]; the host sums the 8 partials.

Per-core layout strategy:
  - QKV proj: out[seq, :] tiles via lhsT = hsT k-tile (stationary), rhs = W.T.
  - RoPE in [seq, d] layout (free-dim rotate-half), 1/sqrt(HD) folded into
    the q cos/sin tables.
  - q/k transposed to [d, seq] via PE transpose (fp32), cast bf16.
  - scores computed TRANSPOSED: scoresT[kv, seq] = K_T_tile.T @ qT. Softmax
    runs without max-subtraction (constant -20 shift cancels per-row; exp
    range is safe for this regime), so it is single-pass and P never needs
    transposing. Denominator = DVE accumulation over kv tiles + ones-matmul
    partition reduce.
  - attn: attnT[d, seq] accumulated via lhsT = v_kt [kv, d], rhs = pT_kt.
  - 1/denom applied after attn via a K=1 broadcast matmul + DVE multiply.
  - o_proj: out[seq, hid] tiles, lhsT = attnT head-slice, 4-head accumulate.
"""

import math
import numpy as np
import ml_dtypes

import bass_rust
import concourse.bass as bass
import concourse.mybir as mybir
import concourse.tile as tile
from concourse.vector_clock import ScopedClock
from concourse.masks import make_identity
from concourse.bass_utils import run_bass_kernel_spmd

# ---------------------------------------------------------------------------
# Workaround: walrus in this image rejects >1 sem wait on CTRL-class
# instructions (Drain/NoOp). TileContext's tail drain waits on every touched
# logical processor. Split the waits across preceding sync-engine nops.
MAX_WAITS = 1


def _split_waits(nc, inst):
    si = inst.ins.sync_info
    if si is None:
        return
    waits = list(si.on_wait)
    if len(waits) <= MAX_WAITS:
        return
    inst.ins.sync_info = bass_rust.SyncInfo(
        on_wait=waits[:MAX_WAITS], on_update=list(si.on_update)
    )
    rest = waits[MAX_WAITS:]
    while rest:
        extra = nc.sync.nop(nofuse=True)
        extra.ins.sync_info = bass_rust.SyncInfo(on_wait=rest[:MAX_WAITS], on_update=[])
        rest = rest[MAX_WAITS:]


def _drain_and_barrier_split(self, tick_clock, wait_clock):
    nc = self.nc
    carrier = nc.sync.nop(nofuse=True)
    wait_clock.add_sem_waits(carrier.ins, ScopedClock({None: tick_clock.global_clock}))
    _split_waits(nc, carrier)
    nc.sync.drain()
    nc.all_engine_barrier()
    popped = nc._tile_sem_poison_stack.pop()
    assert popped is self._sem_poison
    nc.clear_and_free_semaphores(list(self.sems.allocated().values()))
    nc.all_engine_barrier()


tile.TileContext._drain_and_barrier = _drain_and_barrier_split
# ---------------------------------------------------------------------------

# ---------------------------------------------------------------------------
# General wait-cap legalization: this walrus rejects instructions carrying
# more than a couple of sem waits. Post-process the BIR JSON: hoist overflow
# waits onto engine-matched NoOps inserted immediately before the offender
# (same engine queue -> same ordering semantics).
import json as _json

_CTRL_OPS = {"NoOp", "Drain", "EventSemaphore"}
_CAP_CTRL = 1
_CAP_OTHER = 1
_orig_to_json_bytes = bass.Bass.to_json_bytes


def _legalized_to_json_bytes(self, *a, **k):
    raw = _orig_to_json_bytes(self, *a, **k)
    m = _json.loads(raw)
    ctr = [0]
    changed = False
    for fn in m.get("functions", []):
        for blk in fn.get("blocks", []):
            insts = blk.get("instructions", [])
            out = []
            for ins in insts:
                si = ins.get("sync_info")
                if si:
                    waits = si.get("on_wait") or []
                    cap = _CAP_CTRL if ins.get("opcode") in _CTRL_OPS else _CAP_OTHER
                    if len(waits) > cap:
                        changed = True
                        rest = waits[:-cap]
                        si["on_wait"] = waits[-cap:]
                        while rest:
                            ctr[0] += 1
                            out.append({
                                "debug": ins.get("debug", 0),
                                "engine": ins["engine"],
                                "ins": [], "outs": [],
                                "name": f"{ins['name']}_lw{ctr[0]}",
                                "opcode": "NoOp",
                                "sync_info": {"on_wait": rest[:_CAP_CTRL],
                                              "on_update": []},
                            })
                            rest = rest[_CAP_CTRL:]
                out.append(ins)
            blk["instructions"] = out
    if not changed:
        return raw
    return _json.dumps(m).encode()


bass.Bass.to_json_bytes = _legalized_to_json_bytes
# ---------------------------------------------------------------------------


B, Q, PAST, HID = 1, 1024, 3072, 4096
NH, NKV, HD = 32, 8, 128
KV = PAST + Q           # 4096
NCORES = 8
HPC = NH // NCORES      # 4 query heads per core
ROPE_THETA = 10000.0
EXP_SHIFT = -20.0       # constant softmax shift (cancels exactly per row)

F32 = mybir.dt.float32
BF16 = mybir.dt.bfloat16

N_KT = KV // 128        # 32 kv tiles
N_ST = Q // 128         # 8 seq tiles
N_HK = HID // 128       # 32 hid k-tiles
GRP = 512               # seq group width for scores/attn
N_G = Q // GRP          # 2 groups
N_PV = PAST // 128      # 24 past-v tiles

LAST_RESULTS = None     # test harness reads exec_time_ns from here


def _build_program():
    nc = bass.Bass()
    hst = nc.declare_dram_parameter("hst", [128, N_HK * Q], BF16, isOutput=False)
    wqt = nc.declare_dram_parameter("wqt", [128, N_HK * HPC * 128], BF16, isOutput=False)
    wkvt = nc.declare_dram_parameter("wkvt", [128, N_HK * 256], BF16, isOutput=False)
    pastkt = nc.declare_dram_parameter("pastkt", [128, PAST], BF16, isOutput=False)
    pastv = nc.declare_dram_parameter("pastv", [128, PAST], BF16, isOutput=False)
    maskt = nc.declare_dram_parameter("maskt", [128, N_KT * Q], BF16, isOutput=False)
    cosq = nc.declare_dram_parameter("cosq", [128, N_ST * HD], F32, isOutput=False)
    sinq = nc.declare_dram_parameter("sinq", [128, N_ST * HD], F32, isOutput=False)
    cosk = nc.declare_dram_parameter("cosk", [128, N_ST * HD], F32, isOutput=False)
    sink = nc.declare_dram_parameter("sink", [128, N_ST * HD], F32, isOutput=False)
    wot = nc.declare_dram_parameter("wot", [128, HPC * HID], BF16, isOutput=False)
    outp = nc.declare_dram_parameter("outp", [Q, HID], F32, isOutput=True)

    with tile.TileContext(nc) as tc:
        with (
            tc.tile_pool(name="const", bufs=1) as cpool,
            tc.tile_pool(name="kvres", bufs=1) as kvpool,
            tc.tile_pool(name="qt", bufs=1) as qtpool,
            tc.tile_pool(name="attn", bufs=1) as apool,
        ):
            ident = cpool.tile([128, 128], F32)
            make_identity(nc, ident[:])
            ones_col = cpool.tile([128, 1], F32)
            nc.vector.memset(ones_col[:], 1.0)
            ones_row = cpool.tile([1, 128], F32)
            nc.vector.memset(ones_row[:], 1.0)
            shift_sb = cpool.tile([128, 1], F32)
            nc.vector.memset(shift_sb[:], EXP_SHIFT)

            # K_T [128 d, KV] bf16; V packed [128 kv-sub, N_KT*128 d]
            kt_sb = kvpool.tile([128, KV], BF16)
            nc.sync.dma_start(kt_sb[:, :PAST], pastkt[:])
            v_sb = kvpool.tile([128, N_KT * 128], BF16)
            nc.sync.dma_start(v_sb[:, : N_PV * 128], pastv[:])

            # qT per head [128 d, Q] bf16; attnT per head [128 d, Q] bf16
            qt_sb = [qtpool.tile([128, Q], BF16, tag=f"qt{h}", name=f"qt{h}") for h in range(HPC)]
            at_sb = [apool.tile([128, Q], BF16, tag=f"at{h}", name=f"at{h}") for h in range(HPC)]

            # ---------------- stage 1: QKV projection + RoPE ----------------
            with (
                tc.tile_pool(name="hsw", bufs=1) as hspool,
                tc.tile_pool(name="rope", bufs=3) as rpool,
                tc.tile_pool(name="tps", bufs=2, space="PSUM") as tps,
                tc.tile_pool(name="qkvps", bufs=2, space="PSUM") as qkvps,
            ):
                cosq_sb = hspool.tile([128, N_ST * HD], F32)
                sinq_sb = hspool.tile([128, N_ST * HD], F32)
                cosk_sb = hspool.tile([128, N_ST * HD], F32)
                sink_sb = hspool.tile([128, N_ST * HD], F32)
                nc.sync.dma_start(cosq_sb[:], cosq[:])
                nc.sync.dma_start(sinq_sb[:], sinq[:])
                nc.sync.dma_start(cosk_sb[:], cosk[:])
                nc.sync.dma_start(sink_sb[:], sink[:])

                hs_sb = hspool.tile([128, N_HK * Q], BF16)
                wq_sb = hspool.tile([128, N_HK * HPC * 128], BF16)
                wkv_sb = hspool.tile([128, N_HK * 256], BF16)
                for i in range(8):
                    s, e = i * (N_HK // 8), (i + 1) * (N_HK // 8)
                    nc.sync.dma_start(hs_sb[:, s * Q:e * Q], hst[:, s * Q:e * Q])
                    nc.sync.dma_start(
                        wq_sb[:, s * HPC * 128:e * HPC * 128],
                        wqt[:, s * HPC * 128:e * HPC * 128],
                    )
                nc.sync.dma_start(wkv_sb[:], wkvt[:])

                def rope(dst_bf, src_ps, cos_t, sin_t, st):
                    """dst_bf [128 d, 128 seq] (TRANSPOSED) <- RoPE(src [seq,d]).

                    sin tables have cols 0:64 pre-negated:
                      rot[:, :64] = src[:, 64:] * sin[:, :64]
                      rot[:, 64:] = src[:, :64] * sin[:, 64:]
                    """
                    c = cos_t[:, st * HD:(st + 1) * HD]
                    s = sin_t[:, st * HD:(st + 1) * HD]
                    out_f = rpool.tile([128, HD], F32, tag="ropeout")
                    nc.vector.tensor_mul(out_f[:, 0:64], src_ps[:, 64:128], s[:, 0:64])
                    nc.vector.tensor_mul(out_f[:, 64:128], src_ps[:, 0:64], s[:, 64:128])
                    cos_part = rpool.tile([128, HD], F32, tag="tcos")
                    nc.vector.tensor_mul(cos_part[:], src_ps[:], c)
                    nc.vector.tensor_add(out_f[:], out_f[:], cos_part[:])
                    tp = tps.tile([128, HD], F32, tag="tp")
                    nc.tensor.transpose(tp[:], out_f[:], ident[:])
                    nc.vector.tensor_copy(dst_bf, tp[:])

                for st in range(N_ST):
                    q_ps = qkvps.tile([128, HPC * 128], F32, tag="qps")
                    kv_ps = qkvps.tile([128, 256], F32, tag="kvps")
                    for k in range(N_HK):
                        lhs = hs_sb[:, k * Q + st * 128: k * Q + (st + 1) * 128]
                        nc.tensor.matmul(
                            q_ps[:], lhs,
                            wq_sb[:, k * HPC * 128:(k + 1) * HPC * 128],
                            start=(k == 0), stop=(k == N_HK - 1),
                        )
                        nc.tensor.matmul(
                            kv_ps[:], lhs, wkv_sb[:, k * 256:(k + 1) * 256],
                            start=(k == 0), stop=(k == N_HK - 1),
                        )
                    for h in range(HPC):
                        rope(qt_sb[h][:, st * 128:(st + 1) * 128],
                             q_ps[:, h * 128:(h + 1) * 128], cosq_sb, sinq_sb, st)
                    rope(kt_sb[:, PAST + st * 128: PAST + (st + 1) * 128],
                         kv_ps[:, 0:128], cosk_sb, sink_sb, st)
                    nc.vector.tensor_copy(
                        v_sb[:, (N_PV + st) * 128:(N_PV + st + 1) * 128],
                        kv_ps[:, 128:256],
                    )

            # ---------------- stage 2: attention ----------------
            with (
                tc.tile_pool(name="mask", bufs=2) as mpool,
                tc.tile_pool(name="sc", bufs=4) as scpool,
                tc.tile_pool(name="pt", bufs=6) as ptpool,
                tc.tile_pool(name="dn", bufs=3) as dnpool,
                tc.tile_pool(name="scps", bufs=3, space="PSUM") as scps,
                tc.tile_pool(name="aps", bufs=2, space="PSUM") as aps,
                tc.tile_pool(name="dps", bufs=1, space="PSUM") as dps,
            ):
                for g in range(N_G):
                    gsl = slice(g * GRP, (g + 1) * GRP)
                    # mask tiles for this group, reused by all 4 heads
                    m_sb = mpool.tile([128, N_KT * GRP], BF16, tag="mask")
                    for kt in range(N_KT):
                        nc.sync.dma_start(
                            m_sb[:, kt * GRP:(kt + 1) * GRP],
                            maskt[:, kt * Q + g * GRP: kt * Q + (g + 1) * GRP],
                        )
                    for h in range(HPC):
                        a_ps = aps.tile([128, GRP], F32, tag="aacc")
                        dn_sb = dnpool.tile([128, GRP], F32, tag="dpart")
                        for kt in range(N_KT):
                            s_ps = scps.tile([128, GRP], F32, tag="sps")
                            nc.tensor.matmul(
                                s_ps[:], kt_sb[:, kt * 128:(kt + 1) * 128],
                                qt_sb[h][:, gsl], start=True, stop=True,
                            )
                            s_sb = scpool.tile([128, GRP], F32, tag="ssb")
                            nc.vector.tensor_add(
                                s_sb[:], s_ps[:], m_sb[:, kt * GRP:(kt + 1) * GRP]
                            )
                            pt = ptpool.tile([128, GRP], BF16, tag="pt")
                            nc.scalar.activation(
                                pt[:], s_sb[:],
                                mybir.ActivationFunctionType.Exp,
                                bias=shift_sb[:], scale=1.0,
                            )
                            if kt == 0:
                                nc.vector.tensor_copy(dn_sb[:], pt[:])
                            else:
                                nc.vector.tensor_add(dn_sb[:], dn_sb[:], pt[:])
                            nc.tensor.matmul(
                                a_ps[:], v_sb[:, kt * 128:(kt + 1) * 128], pt[:],
                                start=(kt == 0), stop=(kt == N_KT - 1),
                            )
                        # denominator: partition-reduce then broadcast 1/denom
                        ds_ps = dps.tile([1, GRP], F32, tag="dsum")
                        nc.tensor.matmul(ds_ps[:], ones_col[:], dn_sb[:],
                                         start=True, stop=True)
                        rc_sb = dnpool.tile([1, GRP], F32, tag="recip")
                        nc.vector.reciprocal(rc_sb[:], ds_ps[:])
                        bc_ps = dps.tile([128, GRP], F32, tag="bcast")
                        nc.tensor.matmul(bc_ps[:], ones_row[:], rc_sb[:],
                                         start=True, stop=True)
                        bc_sb = dnpool.tile([128, GRP], F32, tag="bcsb")
                        nc.vector.tensor_copy(bc_sb[:], bc_ps[:])
                        nc.vector.tensor_mul(at_sb[h][:, gsl], a_ps[:], bc_sb[:])

            # ---------------- stage 3: o_proj partial ----------------
            with (
                tc.tile_pool(name="wo", bufs=1) as wopool,
                tc.tile_pool(name="ostage", bufs=4) as ostpool,
                tc.tile_pool(name="ops", bufs=4, space="PSUM") as opps,
            ):
                wo_sb = wopool.tile([128, HPC * HID], BF16)
                for h in range(HPC):
                    nc.sync.dma_start(
                        wo_sb[:, h * HID:(h + 1) * HID],
                        wot[:, h * HID:(h + 1) * HID],
                    )
                for st in range(N_ST):
                    for n in range(HID // 512):
                        o_ps = opps.tile([128, 512], F32, tag="ops")
                        for h in range(HPC):
                            nc.tensor.matmul(
                                o_ps[:],
                                at_sb[h][:, st * 128:(st + 1) * 128],
                                wo_sb[:, h * HID + n * 512: h * HID + (n + 1) * 512],
                                start=(h == 0), stop=(h == HPC - 1),
                            )
                        o_sb = ostpool.tile([128, 512], F32, tag="osb")
                        nc.vector.tensor_copy(o_sb[:], o_ps[:])
                        nc.sync.dma_start(
                            outp[st * 128:(st + 1) * 128, n * 512:(n + 1) * 512],
                            o_sb[:],
                        )
    return nc


def _pack_ktiles(a, tile_rows=128):
    """[R, C] -> [128, (R//128)*C] with k-tile kt at cols [kt*C:(kt+1)*C]."""
    r, c = a.shape
    n = r // tile_rows
    return np.ascontiguousarray(
        a.reshape(n, tile_rows, c).transpose(1, 0, 2).reshape(tile_rows, n * c)
    )


def _rope_tables(position_ids):
    pos = np.asarray(position_ids).reshape(-1).astype(np.float64)
    inv_freq = 1.0 / (ROPE_THETA ** (np.arange(0, HD, 2, dtype=np.float64) / HD))
    freqs = np.outer(pos, inv_freq)                      # [Q, 64]
    emb = np.concatenate([freqs, freqs], axis=-1)        # [Q, HD]
    return np.cos(emb).astype(np.float32), np.sin(emb).astype(np.float32)


def kernel(hidden_states, attention_mask, position_ids, past_k, past_v,
           Wq, Wk, Wv, Wo):
    global LAST_RESULTS
    bf = ml_dtypes.bfloat16

    hs = np.asarray(hidden_states, np.float32).reshape(Q, HID)
    mask = np.asarray(attention_mask, np.float32).reshape(Q, KV)
    cos, sin = _rope_tables(position_ids)

    scale = 1.0 / math.sqrt(HD)
    # sin tables: cols 0:64 negated (rotate-half sign), q tables pre-scaled
    sin_eff = sin.copy()
    sin_eff[:, :64] = -sin_eff[:, :64]
    cosq_t = _pack_ktiles(cos * scale)
    sinq_t = _pack_ktiles(sin_eff * scale)
    cosk_t = _pack_ktiles(cos)
    sink_t = _pack_ktiles(sin_eff)

    hst = _pack_ktiles(np.ascontiguousarray(hs.T)).astype(bf)      # [128, 32*1024]
    maskt = _pack_ktiles(np.ascontiguousarray(mask.T)).astype(bf)  # [128, 32*1024]

    nc = _build_program()
    in_maps = []
    for c in range(NCORES):
        qs = slice(c * HPC * HD, (c + 1) * HPC * HD)
        ks = slice(c * HD, (c + 1) * HD)
        wq_c = _pack_ktiles(np.ascontiguousarray(Wq[qs, :].T)).astype(bf)
        wk_c = np.ascontiguousarray(Wk[ks, :].T)                   # [4096, 128]
        wv_c = np.ascontiguousarray(Wv[ks, :].T)
        wkv_c = _pack_ktiles(
            np.concatenate([wk_c, wv_c], axis=1)).astype(bf)       # [128, 32*256]
        pkt = np.ascontiguousarray(past_k[0, c].T).astype(bf)      # [128, 3072]
        pv = _pack_ktiles(np.ascontiguousarray(past_v[0, c])).astype(bf)
        wo_c = _pack_ktiles(
            np.ascontiguousarray(Wo[:, qs].T)).astype(bf)          # [128, 4*4096]
        in_maps.append({
            "hst": hst, "wqt": wq_c, "wkvt": wkv_c, "pastkt": pkt,
            "pastv": pv, "maskt": maskt, "cosq": cosq_t, "sinq": sinq_t,
            "cosk": cosk_t, "sink": sink_t, "wot": wo_c,
        })

    res = run_bass_kernel_spmd(nc, in_maps, list(range(NCORES)))
    LAST_RESULTS = res
    out = np.zeros((Q, HID), np.float32)
    for c in range(NCORES):
        out += res.results[c]["outp"]
    return out.reshape(B, Q, HID)



# revision 4
# speedup vs baseline: 1.6473x; 1.6473x over previous
"""Llama GQA attention (B=1, Q=1024, PAST=3072, HID=4096, NH=32, NKV=8, HD=128)
tensor-parallel over heads across 8 NeuronCores.

Per core c: kv head c, query heads 4c..4c+3. Each core computes its partial
o_proj contribution [1024, 4096] (fp16); the host sums the 8 partials.

Per-core design (v2 — ACT/PE balanced, DVE offloaded):
  - q/k projected TRANSPOSED: qT[d, seq] = Wslice.T-tiles.T @ hsT k-tiles,
    so no PE transposes for q/k. RoPE runs on [d, seq] PSUM tiles with
    64-partition-shifted DVE ops (rotate-half lives on the partition dim).
    1/sqrt(HD) is folded into Wq host-side so q and k share rope tables.
  - v projected transposed too, then PE-transposed per 128-tile to [kv, d].
  - attention in 4 passes of (group g, head-pair): per kv tile kt one
    [128,1024] f32 PSUM scores tile (2 heads), ONE exp activation over it,
    fp16 P. Fully-masked (g,kt) tiles are skipped; boundary tiles multiply
    a 0/1 fp16 mask after exp. Denominator accumulated on DVE in fp16
    (2x mode), partition-reduced by a ones-matmul, reciprocal on ACT,
    broadcast back by a K=1 matmul.
  - heads 2,3 projection matmuls are interleaved ("pumped") into passes 1-2
    to fill PE gaps while ACT runs exp.
  - o_proj: out[seq, hid] tiles, lhsT = attnT head-slice, 4-head PSUM
    accumulation, fp16 partial DMA'd out; host sums cores in f32.
"""

import math
import numpy as np

import bass_rust
import concourse.bass as bass
import concourse.mybir as mybir
import concourse.tile as tile
from concourse.vector_clock import ScopedClock
from concourse.masks import make_identity
from concourse.bass_utils import run_bass_kernel_spmd

# ---------------------------------------------------------------------------
# Workaround: walrus in this image rejects >1 sem wait on CTRL-class
# instructions (Drain/NoOp). TileContext's tail drain waits on every touched
# logical processor. Split the waits across preceding sync-engine nops.
MAX_WAITS = 1


def _split_waits(nc, inst):
    si = inst.ins.sync_info
    if si is None:
        return
    waits = list(si.on_wait)
    if len(waits) <= MAX_WAITS:
        return
    inst.ins.sync_info = bass_rust.SyncInfo(
        on_wait=waits[:MAX_WAITS], on_update=list(si.on_update)
    )
    rest = waits[MAX_WAITS:]
    while rest:
        extra = nc.sync.nop(nofuse=True)
        extra.ins.sync_info = bass_rust.SyncInfo(on_wait=rest[:MAX_WAITS], on_update=[])
        rest = rest[MAX_WAITS:]


def _drain_and_barrier_split(self, tick_clock, wait_clock):
    nc = self.nc
    carrier = nc.sync.nop(nofuse=True)
    wait_clock.add_sem_waits(carrier.ins, ScopedClock({None: tick_clock.global_clock}))
    _split_waits(nc, carrier)
    nc.sync.drain()
    nc.all_engine_barrier()
    popped = nc._tile_sem_poison_stack.pop()
    assert popped is self._sem_poison
    nc.clear_and_free_semaphores(list(self.sems.allocated().values()))
    nc.all_engine_barrier()


tile.TileContext._drain_and_barrier = _drain_and_barrier_split
# ---------------------------------------------------------------------------

# ---------------------------------------------------------------------------
# General wait-cap legalization: hoist overflow waits onto engine-matched
# NoOps inserted immediately before the offender.
import json as _json

_CTRL_OPS = {"NoOp", "Drain", "EventSemaphore"}
_CAP_CTRL = 1
_CAP_OTHER = 1
_orig_to_json_bytes = bass.Bass.to_json_bytes


def _legalized_to_json_bytes(self, *a, **k):
    raw = _orig_to_json_bytes(self, *a, **k)
    m = _json.loads(raw)
    ctr = [0]
    changed = False
    for fn in m.get("functions", []):
        for blk in fn.get("blocks", []):
            insts = blk.get("instructions", [])
            out = []
            for ins in insts:
                si = ins.get("sync_info")
                if si:
                    waits = si.get("on_wait") or []
                    cap = _CAP_CTRL if ins.get("opcode") in _CTRL_OPS else _CAP_OTHER
                    if len(waits) > cap:
                        changed = True
                        rest = waits[:-cap]
                        si["on_wait"] = waits[-cap:]
                        while rest:
                            ctr[0] += 1
                            out.append({
                                "debug": ins.get("debug", 0),
                                "engine": ins["engine"],
                                "ins": [], "outs": [],
                                "name": f"{ins['name']}_lw{ctr[0]}",
                                "opcode": "NoOp",
                                "sync_info": {"on_wait": rest[:_CAP_CTRL],
                                              "on_update": []},
                            })
                            rest = rest[_CAP_CTRL:]
                out.append(ins)
            blk["instructions"] = out
    if not changed:
        return raw
    return _json.dumps(m).encode()


bass.Bass.to_json_bytes = _legalized_to_json_bytes
# ---------------------------------------------------------------------------


B, Q, PAST, HID = 1, 1024, 3072, 4096
NH, NKV, HD = 32, 8, 128
KV = PAST + Q           # 4096
NCORES = 8
HPC = NH // NCORES      # 4 query heads per core
ROPE_THETA = 10000.0
EXP_SHIFT = -11.0       # constant softmax shift (cancels exactly per row)

F32 = mybir.dt.float32
F16 = mybir.dt.float16
NPF16 = np.float16

N_KT = KV // 128        # 32 kv tiles
N_HK = HID // 128       # 32 hid k-tiles
GRP = 512
N_G = Q // GRP          # 2 groups
N_PV = PAST // 128      # 24 past-v tiles

LAST_RESULTS = None     # test harness reads exec_time_ns from here


def _build_program(kt_lists, boundary, nb):
    """kt_lists[g] = processed kv tiles for group g (fully-masked skipped);
    boundary[(g, kt)] = slot index into the maskb 0/1 tiles."""
    nc = bass.Bass()
    hst = nc.declare_dram_parameter("hst", [128, N_HK * Q], F16, isOutput=False)
    wqt = nc.declare_dram_parameter("wqt", [128, N_HK * HPC * 128], F16, isOutput=False)
    wkvt = nc.declare_dram_parameter("wkvt", [128, N_HK * 256], F16, isOutput=False)
    pastkt = nc.declare_dram_parameter("pastkt", [128, PAST], F16, isOutput=False)
    pastv = nc.declare_dram_parameter("pastv", [128, PAST], F16, isOutput=False)
    cost = nc.declare_dram_parameter("cost", [128, Q], F16, isOutput=False)
    sint = nc.declare_dram_parameter("sint", [128, Q], F16, isOutput=False)
    maskb = nc.declare_dram_parameter("maskb", [128, max(nb, 1) * GRP], F16,
                                      isOutput=False)
    wot = nc.declare_dram_parameter("wot", [128, HPC * HID], F16, isOutput=False)
    outp = nc.declare_dram_parameter("outp", [Q, HID], F16, isOutput=True)

    with tile.TileContext(nc) as tc:
        with (
            tc.tile_pool(name="const", bufs=1) as cpool,
            tc.tile_pool(name="kvres", bufs=1) as kvpool,
            tc.tile_pool(name="qat", bufs=1) as qat,
            tc.tile_pool(name="tbl", bufs=1) as tbl,
            tc.tile_pool(name="ptp", bufs=3) as ptp,
            tc.tile_pool(name="rt", bufs=1) as rt,
        ):
            ident = cpool.tile([128, 128], F16)
            make_identity(nc, ident[:])
            ones_col = cpool.tile([128, 1], F16)
            nc.vector.memset(ones_col[:], 1.0)
            ones_row = cpool.tile([1, 128], F32)
            nc.vector.memset(ones_row[:], 1.0)
            shift_sb = cpool.tile([128, 1], F32)
            nc.vector.memset(shift_sb[:], EXP_SHIFT)
            # warm the exp table set while DMA streams in
            warm_in = cpool.tile([1, 8], F32)
            warm_out = cpool.tile([1, 8], F32)
            nc.vector.memset(warm_in[:], 0.0)
            nc.scalar.activation(warm_out[:], warm_in[:],
                                 mybir.ActivationFunctionType.Exp)

            # K^T [128 d, KV]; V packed [128 kv-sub, kt*128 + d]
            kt_sb = kvpool.tile([128, KV], F16)
            v_sb = kvpool.tile([128, KV], F16)
            nc.sync.dma_start(kt_sb[:, :PAST], pastkt[:])
            nc.sync.dma_start(v_sb[:, : N_PV * 128], pastv[:])

            cos_sb = tbl.tile([128, Q], F16)
            sin_sb = tbl.tile([128, Q], F16)
            nc.sync.dma_start(cos_sb[:], cost[:])
            nc.sync.dma_start(sin_sb[:], sint[:])
            mb_sb = tbl.tile([128, max(nb, 1) * GRP], F16)
            if nb:
                nc.sync.dma_start(mb_sb[:], maskb[:])

            qt = [qat.tile([128, Q], F16, tag=f"qt{h}", name=f"qt{h}") for h in range(HPC)]
            atu = [qat.tile([128, Q], F32, tag=f"au{h}", name=f"au{h}") for h in range(HPC)]
            ats = [qat.tile([128, Q], F16, tag=f"at{h}", name=f"at{h}") for h in range(HPC)]
            dn = [qat.tile([128, GRP], F16, tag=f"dn{i}", name=f"dn{i}") for i in range(2 * HPC)]

            with tc.tile_pool(name="hsw", bufs=1) as hsp:
                hs_sb = hsp.tile([128, N_HK * Q], F16)
                wq_sb = hsp.tile([128, N_HK * HPC * 128], F16)
                wkv_sb = hsp.tile([128, N_HK * 256], F16)
                for k in range(N_HK):
                    nc.sync.dma_start(hs_sb[:, k * Q:(k + 1) * Q],
                                      hst[:, k * Q:(k + 1) * Q])
                for i in range(8):
                    s, e = i * (N_HK // 8), (i + 1) * (N_HK // 8)
                    nc.sync.dma_start(
                        wq_sb[:, s * HPC * 128:e * HPC * 128],
                        wqt[:, s * HPC * 128:e * HPC * 128])
                    nc.sync.dma_start(wkv_sb[:, s * 256:e * 256],
                                      wkvt[:, s * 256:e * 256])

                with tc.tile_pool(name="pps", bufs=2, space="PSUM") as pps:

                    def rope_half(dst, ps, g):
                        """dst [128 d, 512] f16 <- rope(ps [128 d, 512] f32).

                        rows 0:64 of sin_sb are pre-negated:
                          rot[0:64]  = ps[64:128] * sin[0:64]
                          rot[64:128]= ps[0:64]   * sin[64:128]
                        """
                        c = cos_sb[:, g * GRP:(g + 1) * GRP]
                        s = sin_sb[:, g * GRP:(g + 1) * GRP]
                        rot = rt.tile([128, GRP], F32, tag="rot")
                        nc.vector.tensor_mul(rot[0:64, :], ps[64:128, :], s[0:64, :])
                        nc.vector.tensor_mul(rot[64:128, :], ps[0:64, :], s[64:128, :])
                        cb = rt.tile([128, GRP], F32, tag="cb")
                        nc.vector.tensor_mul(cb[:], ps[:], c)
                        nc.vector.tensor_add(dst, rot[:], cb[:])

                    def qk_gen(wslice_fn, dst_fn):
                        """Transposed projection: dst[d, g*512:+512] over 2 groups."""
                        for g in range(N_G):
                            ps = pps.tile([128, GRP], F32, tag="p")
                            for k in range(N_HK):
                                nc.tensor.matmul(
                                    ps[:], wslice_fn(k),
                                    hs_sb[:, k * Q + g * GRP: k * Q + (g + 1) * GRP],
                                    start=(k == 0), stop=(k == N_HK - 1))
                                yield
                            rope_half(dst_fn(g), ps[:], g)

                    def v_gen(vtp):
                        vts = rt.tile([128, Q], F16, tag="vt")
                        for g in range(N_G):
                            ps = pps.tile([128, GRP], F32, tag="p")
                            for k in range(N_HK):
                                nc.tensor.matmul(
                                    ps[:], wkv_sb[:, k * 256 + 128:(k + 1) * 256],
                                    hs_sb[:, k * Q + g * GRP: k * Q + (g + 1) * GRP],
                                    start=(k == 0), stop=(k == N_HK - 1))
                                yield
                            nc.vector.tensor_copy(vts[:, g * GRP:(g + 1) * GRP], ps[:])
                        for st in range(Q // 128):
                            tp = vtp.tile([128, 128], F16, tag="vtp")
                            nc.tensor.transpose(
                                tp[:], vts[:, st * 128:(st + 1) * 128], ident[:])
                            yield
                            nc.vector.tensor_copy(
                                v_sb[:, (N_PV + st) * 128:(N_PV + st + 1) * 128],
                                tp[:])

                    def pump(gens, n):
                        done = 0
                        while gens and done < n:
                            try:
                                next(gens[0])
                                done += 1
                            except StopIteration:
                                gens.pop(0)

                    def q_w(h):
                        return lambda k: wq_sb[:, k * HPC * 128 + h * 128:
                                               k * HPC * 128 + (h + 1) * 128]

                    def q_dst(h):
                        return lambda g: qt[h][:, g * GRP:(g + 1) * GRP]

                    # ---- projection head: k, v, q0, q1 fully; q2, q3 pumped ----
                    with tc.tile_pool(name="vtp", bufs=2, space="PSUM") as vtp:
                        head = [
                            qk_gen(lambda k: wkv_sb[:, k * 256:k * 256 + 128],
                                   lambda g: kt_sb[:, PAST + g * GRP:
                                                   PAST + (g + 1) * GRP]),
                            v_gen(vtp),
                            qk_gen(q_w(0), q_dst(0)),
                            qk_gen(q_w(1), q_dst(1)),
                        ]
                        pump(head, 10 ** 9)

                    pending = [qk_gen(q_w(2), q_dst(2)), qk_gen(q_w(3), q_dst(3))]

                    # ---- attention passes ----
                    with (
                        tc.tile_pool(name="scp", bufs=2, space="PSUM") as scp,
                        tc.tile_pool(name="att", bufs=2, space="PSUM") as att,
                    ):
                        for (g, ha, hb) in ((0, 0, 1), (1, 0, 1), (0, 2, 3),
                                            (1, 2, 3)):
                            if ha == 2:
                                pump(pending, 10 ** 9)  # p3/p4 need q2/q3 done
                            kts = kt_lists[g]
                            acc = [att.tile([128, GRP], F32, tag="acc",
                                             name=f"acc{g}_{ha}_{jj}")
                                   for jj in range(2)]
                            for i, kt in enumerate(kts):
                                s_ps = scp.tile([128, 2 * GRP], F32, tag="sc")
                                for j, hh in enumerate((ha, hb)):
                                    nc.tensor.matmul(
                                        s_ps[:, j * GRP:(j + 1) * GRP],
                                        kt_sb[:, kt * 128:(kt + 1) * 128],
                                        qt[hh][:, g * GRP:(g + 1) * GRP],
                                        start=True, stop=True)
                                pump(pending, 3)
                                pt = ptp.tile([128, 2 * GRP], F16, tag="pt")
                                nc.scalar.activation(
                                    pt[:], s_ps[:],
                                    mybir.ActivationFunctionType.Exp,
                                    bias=shift_sb[:], scale=1.0)
                                sl = boundary.get((g, kt))
                                if sl is not None:
                                    for j in range(2):
                                        nc.vector.tensor_mul(
                                            pt[:, j * GRP:(j + 1) * GRP],
                                            pt[:, j * GRP:(j + 1) * GRP],
                                            mb_sb[:, sl * GRP:(sl + 1) * GRP])
                                for j, hh in enumerate((ha, hb)):
                                    half = pt[:, j * GRP:(j + 1) * GRP]
                                    d = dn[g * HPC + hh]
                                    if i == 0:
                                        nc.vector.tensor_copy(d[:], half)
                                    else:
                                        nc.vector.tensor_add(d[:], d[:], half)
                                    nc.tensor.matmul(
                                        acc[j][:], v_sb[:, kt * 128:(kt + 1) * 128],
                                        half, start=(i == 0),
                                        stop=(i == len(kts) - 1))
                            for j, hh in enumerate((ha, hb)):
                                nc.vector.tensor_copy(
                                    atu[hh][:, g * GRP:(g + 1) * GRP], acc[j][:])
                        pump(pending, 10 ** 9)

            # ---- normalize + o_proj ----
            with (
                tc.tile_pool(name="wo", bufs=1) as wop,
                tc.tile_pool(name="nps", bufs=2, space="PSUM") as nps,
                tc.tile_pool(name="rcp", bufs=2) as rcp,
                tc.tile_pool(name="ops", bufs=4, space="PSUM") as opool,
                tc.tile_pool(name="osb", bufs=4) as osbp,
            ):
                wo_sb = wop.tile([128, HPC * HID], F16)
                for n in range(HID // GRP):
                    for h in range(HPC):
                        nc.sync.dma_start(
                            wo_sb[:, h * HID + n * GRP: h * HID + (n + 1) * GRP],
                            wot[:, h * HID + n * GRP: h * HID + (n + 1) * GRP])

                # 1/D = exp(-ln(D)); all Ln's batched before all Exp's so the
                # ACT table set switches at most twice.
                lnd = [rcp.tile([1, GRP], F32, tag=f"ln{i}", name=f"ln{i}")
                       for i in range(2 * HPC)]
                for i in range(2 * HPC):
                    ds = nps.tile([1, GRP], F32, tag="ds")
                    nc.tensor.matmul(ds[:], ones_col[:], dn[i][:],
                                     start=True, stop=True)
                    nc.scalar.activation(lnd[i][:], ds[:],
                                         mybir.ActivationFunctionType.Ln)
                for g in range(N_G):
                    for h in range(HPC):
                        rc = rcp.tile([1, GRP], F32, tag="rc")
                        nc.scalar.activation(
                            rc[:], lnd[g * HPC + h][:],
                            mybir.ActivationFunctionType.Exp, scale=-1.0)
                        bc = nps.tile([128, GRP], F32, tag="bc")
                        nc.tensor.matmul(bc[:], ones_row[:], rc[:],
                                         start=True, stop=True)
                        nc.vector.tensor_mul(
                            ats[h][:, g * GRP:(g + 1) * GRP],
                            atu[h][:, g * GRP:(g + 1) * GRP], bc[:])

                for n in range(HID // GRP):
                    for st in range(Q // 128):
                        o_ps = opool.tile([128, GRP], F32, tag="o")
                        for h in range(HPC):
                            nc.tensor.matmul(
                                o_ps[:], ats[h][:, st * 128:(st + 1) * 128],
                                wo_sb[:, h * HID + n * GRP: h * HID + (n + 1) * GRP],
                                start=(h == 0), stop=(h == HPC - 1))
                        ob = osbp.tile([128, GRP], F16, tag="ob")
                        nc.vector.tensor_copy(ob[:], o_ps[:])
                        nc.sync.dma_start(
                            outp[st * 128:(st + 1) * 128, n * GRP:(n + 1) * GRP],
                            ob[:])
    return nc


def _pack_ktiles(a, tile_rows=128):
    """[R, C] -> [128, (R//128)*C] with k-tile kt at cols [kt*C:(kt+1)*C]."""
    r, c = a.shape
    n = r // tile_rows
    return np.ascontiguousarray(
        a.reshape(n, tile_rows, c).transpose(1, 0, 2).reshape(tile_rows, n * c)
    )


def _rope_tables_T(position_ids):
    """cos/sin tables in [d, seq] layout; sin rows 0:64 pre-negated."""
    pos = np.asarray(position_ids).reshape(-1).astype(np.float64)
    inv_freq = 1.0 / (ROPE_THETA ** (np.arange(0, HD, 2, dtype=np.float64) / HD))
    freqs = np.outer(pos, inv_freq)                      # [Q, 64]
    emb = np.concatenate([freqs, freqs], axis=-1)        # [Q, HD]
    cosT = np.cos(emb).T.astype(np.float32)              # [128, Q]
    sinT = np.sin(emb).T.astype(np.float32)
    sinT[:64, :] = -sinT[:64, :]
    return cosT, sinT


def kernel(hidden_states, attention_mask, position_ids, past_k, past_v,
           Wq, Wk, Wv, Wo):
    global LAST_RESULTS

    hs = np.asarray(hidden_states, np.float32).reshape(Q, HID)
    mask = np.asarray(attention_mask, np.float32).reshape(Q, KV)
    cosT, sinT = _rope_tables_T(position_ids)

    # classify (g, kt) tiles from the additive mask
    keep = mask > -1e8                                   # [Q, KV] True=attend
    kt_lists = []
    boundary = {}
    mtiles = []
    for g in range(N_G):
        lst = []
        for kt in range(N_KT):
            blk = keep[g * GRP:(g + 1) * GRP, kt * 128:(kt + 1) * 128]
            if not blk.any():
                continue
            lst.append(kt)
            if not blk.all():
                boundary[(g, kt)] = len(mtiles)
                mtiles.append(np.ascontiguousarray(blk.T).astype(NPF16))
        kt_lists.append(lst)
    nb = len(mtiles)
    maskb = (np.concatenate(mtiles, axis=1) if nb
             else np.zeros((128, GRP), NPF16))

    scale = 1.0 / math.sqrt(HD)
    hst = _pack_ktiles(np.ascontiguousarray(hs.T)).astype(NPF16)  # [128, 32*1024]

    nc = _build_program(kt_lists, boundary, nb)
    in_maps = []
    for c in range(NCORES):
        qs = slice(c * HPC * HD, (c + 1) * HPC * HD)
        ks = slice(c * HD, (c + 1) * HD)
        wq_c = _pack_ktiles(
            np.ascontiguousarray((Wq[qs, :] * scale).T)).astype(NPF16)
        wk_c = np.ascontiguousarray(Wk[ks, :].T)                   # [4096, 128]
        wv_c = np.ascontiguousarray(Wv[ks, :].T)
        wkv_c = _pack_ktiles(
            np.concatenate([wk_c, wv_c], axis=1)).astype(NPF16)    # [128, 32*256]
        pkt = np.ascontiguousarray(past_k[0, c].T).astype(NPF16)   # [128, 3072]
        pv = _pack_ktiles(np.ascontiguousarray(past_v[0, c])).astype(NPF16)
        wo_c = _pack_ktiles(
            np.ascontiguousarray(Wo[:, qs].T)).astype(NPF16)       # [128, 4*4096]
        in_maps.append({
            "hst": hst, "wqt": wq_c, "wkvt": wkv_c, "pastkt": pkt,
            "pastv": pv, "cost": cosT.astype(NPF16),
            "sint": sinT.astype(NPF16), "maskb": maskb, "wot": wo_c,
        })

    res = run_bass_kernel_spmd(nc, in_maps, list(range(NCORES)))
    LAST_RESULTS = res
    out = np.zeros((Q, HID), np.float32)
    for c in range(NCORES):
        out += res.results[c]["outp"].astype(np.float32)
    return out.reshape(B, Q, HID)
